# revision 27
# baseline (speedup 1.0000x reference)
"""Trainium2 Bass kernel for nn_Attention_16612933500996 (v2: fp8 DoubleRow).

Full-input contract: kernel(**inputs) takes the unsharded inputs and returns
the full output. Internally shards across 8 NeuronCores: core i handles
batch b = i//2 and query-half w = i%2 (1024 of 2048 tokens). No collectives:
each core recomputes K/V for its whole batch (x rows are rotated host-side so
each core's query tokens are always rows 0..1023 — softmax over keys is
permutation invariant).

v2 changes over v1 (551us):
  * All projection matmuls (V/K/Q), the U (att @ V) matmul and the output
    projection run in fp8e4 with perf_mode=DoubleRow: 2 fp8 weights/cell
    virtualize the PE array to 128x256, halving streaming time per
    contraction row (HW-verified 237ns vs 292ns per F=512 matmul at 2x K).
    Numpy end-to-end sim of all fp8 casts: rel err 4.2e-3 (budget 2e-2).
  * exp writes fp8e4 directly (ACT is 1x rate regardless of dtype); U
    consumes exp pairs [128,2,512] + vaug pairs [128,2,65] per DoubleRow
    matmul (contraction = 256 keys/pass).
  * The per-head 1/S smalls (Ln + Exp on [1,1024], 2.3us/head of ACT) are
    batched in groups of 4 heads: S rows are gathered by SBUF->SBUF DMA into
    [4,1024], one Ln + one Exp cover 4 heads, result DMA'd back to a flat
    [1,4096] row for the per-head broadcast matmuls. Saves ~27us of ACT
    stream time in the exp phase.
  * Output tiles leave as bf16 (host upcasts); halves the tail DMA.
"""

import sys

sys.path.insert(0, "/opt/trn_rl_repo")

import numpy as np
import ml_dtypes

import concourse.bass as bass
import concourse.tile as tile
from concourse import mybir
from concourse.bass_utils import run_bass_kernel_spmd

AF = mybir.ActivationFunctionType
ALU = mybir.AluOpType
PM = mybir.MatmulPerfMode
F32 = mybir.dt.float32
BF16 = mybir.dt.bfloat16
F8E4 = mybir.dt.float8e4

B, L, D = 4, 2048, 1024
H, HD = 16, 64
WQ = 1024          # query tokens per core
N_CORES = 8
SCALE = 1.0 / float(np.sqrt(np.float32(L)))
LN_EPS = 1e-5
BF = ml_dtypes.bfloat16
F8 = ml_dtypes.float8_e4m3


def _patch_tile_drain():
    """walrus in this container only accepts 1 sem wait on the TPB_CTRL drain;
    split the TileContext tail-drain waits across multiple drain instructions."""
    if getattr(tile.TileContext, "_drain_patched", False):
        return
    from concourse.tile import ScopedClock

    def _drain_and_barrier(self, tick_clock, wait_clock):
        nc = self.nc
        drain_inst = nc.sync.drain()
        wait_clock.add_sem_waits(
            drain_inst.ins, ScopedClock({None: tick_clock.global_clock})
        )
        si = drain_inst.ins.sync_info
        waits = list(si.on_wait) if si is not None else []
        MAXW = 1
        if len(waits) > MAXW:
            drain_inst.ins.sync_info = mybir.SyncInfo(
                on_wait=waits[:MAXW], on_update=list(si.on_update)
            )
            for i in range(MAXW, len(waits), MAXW):
                d2 = nc.sync.drain()
                d2.ins.sync_info = mybir.SyncInfo(
                    on_wait=waits[i : i + MAXW], on_update=[]
                )
        nc.all_engine_barrier()
        popped = nc._tile_sem_poison_stack.pop()
        assert popped is self._sem_poison
        nc.clear_and_free_semaphores(list(self.sems.allocated().values()))
        nc.all_engine_barrier()

    tile.TileContext._drain_and_barrier = _drain_and_barrier
    tile.TileContext._drain_patched = True


def _split_excess_waits(nc, max_waits=1):
    """walrus in this container has a tight per-instruction sync-wait slot
    limit; move excess waits onto same-engine nops preceding the instruction
    (same-engine queue order makes sequential waiting equivalent)."""
    for f in nc.m.functions:
        for bb in f.blocks:
            out = []
            changed = False
            for inst in bb.instructions:
                si = inst.sync_info
                waits = list(si.on_wait) if si is not None else []
                if len(waits) > max_waits:
                    lead = waits[: len(waits) - max_waits]
                    keep = waits[len(waits) - max_waits :]
                    for i in range(0, len(lead), max_waits):
                        nop = mybir.InstNoOp(
                            name=f"{inst.name}_w{i}", engine=inst.engine, ins=[], outs=[]
                        )
                        nop.sync_info = mybir.SyncInfo(
                            on_wait=lead[i : i + max_waits], on_update=[]
                        )
                        out.append(nop)
                    inst.sync_info = mybir.SyncInfo(
                        on_wait=keep, on_update=list(si.on_update)
                    )
                    changed = True
                out.append(inst)
            if changed:
                bb.instructions = out


def build_program(split_waits=True):
    _patch_tile_drain()
    nc = bass.Bass("TRN2", target_bir_lowering=False, debug=False, num_devices=N_CORES)

    x8_d = nc.dram_tensor("x8", [128, 8 * L], F8E4, kind="ExternalInput")
    xq_d = nc.dram_tensor("xq", [WQ, D], F32, kind="ExternalInput")
    wqk_d = nc.dram_tensor("wqk", [16, 128, 1024], F8E4, kind="ExternalInput")
    wv_d = nc.dram_tensor("wv", [128, 8 * 1024], F8E4, kind="ExternalInput")
    w2_d = nc.dram_tensor("w2", [128, 8 * 1024], F8E4, kind="ExternalInput")
    bqk_d = nc.dram_tensor("bqk", [128, 16], F32, kind="ExternalInput")
    bv_d = nc.dram_tensor("bv", [1, H * HD], BF16, kind="ExternalInput")
    b2_d = nc.dram_tensor("b2", [1, D], BF16, kind="ExternalInput")
    out_d = nc.dram_tensor("out", [WQ, D], BF16, kind="ExternalOutput")

    NT = L // 128            # 16 token tiles
    ND = D // 128            # 8 d tiles
    NW = WQ // 128           # 8 query-token tiles
    NM = L // 128            # 16 key tiles
    NP = NM // 2             # 8 key-tile pairs
    GRP = 4                  # heads per 1/S normalization batch

    with tile.TileContext(nc) as tc:
        pers = tc.alloc_tile_pool(name="pers", bufs=1)
        # 3-deep matmul psum ring (6 banks) + single u accumulator (2 banks).
        pmm = tc.alloc_tile_pool(name="pmm", bufs=3, space="PSUM")
        pu = tc.alloc_tile_pool(name="pu", bufs=1, space="PSUM")

        # --- constants ---
        ones = pers.tile([128, 128], BF16, tag="ones")
        nc.gpsimd.memset(ones[:, :], 1.0)
        eps = pers.tile([128, 1], F32, tag="eps")
        nc.gpsimd.memset(eps[:, :], LN_EPS)

        qkv_pool = tc.alloc_tile_pool(name="qkv", bufs=1)
        # q zero-padded per head ([128,WQ], only this head's 64 rows nonzero)
        # so scores use the full-K=128 kT pair as stationary.
        qZ = [qkv_pool.tile([128, WQ], BF16, tag=f"qZ{h}", name=f"qZ{h}") for h in range(H)]
        kT = [qkv_pool.tile([128, L], BF16, tag=f"kT{i}", name=f"kT{i}") for i in range(ND)]
        # vaug pairs: [128 keys, 2 key-tiles, 16 heads, 65 (64 v + ones)] fp8
        vp8 = [qkv_pool.tile([128, 2 * H * 65], F8E4, tag=f"vp{i}", name=f"vp{i}") for i in range(NP)]
        # nv packed for the out-proj DoubleRow: [128 e, 2 e-tiles, WQ] fp8 per pair
        nvP8 = [pers.tile([128, 2 * WQ], F8E4, tag=f"nvp{e}", name=f"nvp{e}") for e in range(ND // 2)]
        w28 = pers.tile([128, 8 * 1024], F8E4, tag="w28")
        b2 = pers.tile([1, D], BF16, tag="b2")
        # S staging: per-head S rows gathered by DMA into [GRP,1024], Ln+Exp,
        # then DMA'd back to a flat [1, GRP*1024] row for broadcast matmuls.
        sT = pers.tile([GRP, 1024], BF16, tag="sT")
        lnT = pers.tile([GRP, 1024], F32, tag="lnT")
        rcbT = pers.tile([GRP, 1024], BF16, tag="rcbT")
        rcbF = pers.tile([1, GRP * 1024], BF16, tag="rcbF")

        for h in range(H):
            nc.vector.memset(qZ[h][:, :], 0.0)

        with tc.tile_pool(name="ph12", bufs=1) as ph1:
            ph2 = ph1
            x8 = ph1.tile([128, 8 * L], F8E4, tag="x8")
            x8v = x8[:, :].rearrange("p (kd t) -> p kd t", kd=8)

            # x8 in column chunks so v-proj starts once chunk 0 lands
            x8dv = x8_d[:, :].rearrange("p (kd t) -> p kd t", kd=8)
            bounds = [0, 128, 384, 768, 1280, 1664, 2048]
            for ch in range(len(bounds) - 1):
                c0, c1 = bounds[ch], bounds[ch + 1]
                nc.sync.dma_start(x8v[:, :, c0:c1], x8dv[:, :, c0:c1])
                if ch == 0:
                    wv8 = ph1.tile([128, 8 * 1024], F8E4, tag="wv8")
                    for kp in range(4):
                        nc.gpsimd.dma_start(
                            wv8[:, kp * 2048 : (kp + 1) * 2048],
                            wv_d[:, kp * 2048 : (kp + 1) * 2048],
                        )
                    bv = ph1.tile([1, H * HD], BF16, tag="bv")
                    nc.gpsimd.dma_start(bv[:, :], bv_d[:, :])
                    bqk = ph1.tile([128, 16], F32, tag="bqk")
                    nc.gpsimd.dma_start(bqk[:, :], bqk_d[:, :])
            wv8v = wv8[:, :].rearrange("p (kd c) -> p kd c", kd=8)

            # q/k projection weights: ring of 4, >=1 head-pair ahead.
            wqk_tiles = {}

            def fetch_wqk(et):
                w = ph1.tile([128, 1024], F8E4, tag="wqk", bufs=4, name=f"wqk{et}")
                nc.sync.dma_start(w[:, :], wqk_d[et, :, :])
                wqk_tiles[et] = w

            for et in (0, 8, 1, 9):
                fetch_wqk(et)

            nc.sync.dma_start(w28[:, :], w2_d[:, :])
            nc.gpsimd.dma_start(b2[:, :], b2_d[:, :])
            w28v = w28[:, :].rearrange("p (e c) -> p e c", e=8)

            # ---- V projection (fp8 DoubleRow, K=256/pass) per token tile.
            # kp outer / c2 inner: one stationary (x-chunk) serves 2 matmuls,
            # halving the un-overlapped 256-col DoubleRow weight loads.
            for ti in range(NT):
                ps = pmm.tile([128, 1024], F32, tag="mm", name=f"vps{ti}")
                for kp in range(4):
                    for c2 in range(2):
                        sl = slice(c2 * 512, (c2 + 1) * 512)
                        mm = nc.tensor.matmul(
                            ps[:, sl],
                            x8v[:, 2 * kp : 2 * kp + 2, ti * 128 : (ti + 1) * 128],
                            wv8v[:, 2 * kp : 2 * kp + 2, sl],
                            start=(kp == 0),
                            stop=False,
                            perf_mode=PM.DoubleRow,
                        )
                        if c2 == 1:
                            mm.ins.ldweights = False
                for c2 in range(2):
                    sl = slice(c2 * 512, (c2 + 1) * 512)
                    mm = nc.tensor.matmul(
                        ps[:, sl],
                        ones[0:1, 0:128],
                        bv[0:1, sl],
                        start=False,
                        stop=True,
                    )
                    if c2 == 1:
                        mm.ins.ldweights = False
                va = vp8[ti // 2]
                va_r = va[:, :].rearrange("p (j h c) -> p j h c", j=2, c=65)
                nc.gpsimd.memset(va_r[:, ti % 2, :, 64:65], 1.0)
                nc.scalar.activation(
                    va_r[:, ti % 2, :, 0:64],
                    ps[:, :],
                    AF.Silu,
                )

            def project_qk(et):
                """q (et<ND) or k (et>=ND) projection, fp8 DoubleRow.
                kp outer / token-chunk inner: one stationary (w-chunk) serves
                all 2 (q) or 4 (k) moving chunks — DoubleRow 256-col weight
                loads don't overlap the running matmul, so reuse them."""
                is_q = et < ND
                qi = et % ND
                wt = wqk_tiles.pop(et)
                wtv = wt[:, :].rearrange("p (kd m) -> p kd m", kd=8)
                bt = bqk[:, et : et + 1]
                ncols = WQ if is_q else L
                nh = ncols // 1024
                # second half from the (idle during proj) pu pool so the
                # pmm ring keeps slots free for cross-et pipelining
                pss = [
                    (pmm if half == 0 else pu).tile(
                        [128, 1024], F32, tag="mm" if half == 0 else "u",
                        name=f"qk{et}_{half}",
                    )
                    for half in range(nh)
                ]
                for kp in range(4):
                    for half in range(nh):
                        for tc2 in range(2):
                            t0 = half * 1024 + tc2 * 512
                            mm = nc.tensor.matmul(
                                pss[half][:, tc2 * 512 : (tc2 + 1) * 512],
                                wtv[:, 2 * kp : 2 * kp + 2, :],
                                x8v[:, 2 * kp : 2 * kp + 2, t0 : t0 + 512],
                                start=(kp == 0),
                                stop=(kp == 3),
                                perf_mode=PM.DoubleRow,
                            )
                            if half + tc2 > 0:
                                mm.ins.ldweights = False
                for half in range(nh):
                    ps = pss[half]
                    if is_q:
                        for pi in range(2):
                            pr = pi * 64
                            nc.scalar.activation(
                                qZ[2 * qi + pi][pr : pr + 64, half * 1024 : (half + 1) * 1024],
                                ps[pr : pr + 64, :],
                                AF.Silu,
                                bias=bt[pr : pr + 64, :],
                            )
                    else:
                        nc.scalar.activation(
                            kT[qi][:, half * 1024 : (half + 1) * 1024],
                            ps[:, :],
                            AF.Silu,
                            bias=bt[:, :],
                        )

            def attn_mms(h, inject):
                """One head: scores (bf16 K=128) -> exp (fp8 out) -> U
                (fp8 DoubleRow over key-tile pairs, fused S row).

                The U matmul for pair mp is issued only after the NEXT pair's
                scores: the PE queue is strict FIFO for matmuls, so an
                early-issued U (waiting on its exps) would head-of-line block
                the following scores and starve the ACT exp stream.

                `inject` maps pair index -> callables to interleave into this
                head's engine streams (the previous group's 1/S work, issued
                late enough that its DMA/ACT deps are long satisfied and the
                queues never stall on it)."""
                et = h // 2
                u = pu.tile([128, 1024], F32, tag="u", name=f"u{h}")

                def issue_u(um, epv):
                    vjv = vp8[um][:, :].rearrange("p (j h c) -> p j h c", j=2, c=65)
                    for wc in range(2):
                        mm = nc.tensor.matmul(
                            u[0:65, wc * 512 : (wc + 1) * 512],
                            vjv[:, :, h, :],
                            epv[:, :, wc * 512 : (wc + 1) * 512],
                            start=(um == 0),
                            stop=(um == NP - 1),
                            perf_mode=PM.DoubleRow,
                        )
                        if wc == 1:
                            mm.ins.ldweights = False

                pend = None
                for mp in range(NP):
                    ep = ph2.tile([128, 2048], F8E4, tag="exp", bufs=2, name=f"ex{h}_{mp}")
                    epv = ep[:, :].rearrange("p (j n) -> p j n", j=2)
                    for j in range(2):
                        mt = 2 * mp + j
                        ps = pmm.tile([128, 1024], F32, tag="mm", name=f"sc{h}_{mt}")
                        for wc in range(2):
                            mm = nc.tensor.matmul(
                                ps[:, wc * 512 : (wc + 1) * 512],
                                kT[et][:, mt * 128 : (mt + 1) * 128],
                                qZ[h][:, wc * 512 : (wc + 1) * 512],
                                start=True,
                                stop=True,
                            )
                            if wc == 1:
                                mm.ins.ldweights = False
                        nc.scalar.activation(
                            ep[:, j * 1024 : (j + 1) * 1024], ps[:, :], AF.Exp, scale=SCALE
                        )
                    if pend is not None:
                        issue_u(*pend)
                    pend = (mp, epv)
                    for fn in inject.get(mp, ()):
                        fn()
                issue_u(*pend)
                return u

            usbs = {}
            uhis = {}
            direct_rcb = {}

            def normalize_a(h, u, stage=True):
                """Free the u psum bank: one bf16 copy of U rows 0-64 + S row.
                Odd heads get their nv rows pre-shuffled to partitions 64:128
                (off the critical path) so the later 1/S multiply can read a
                col-tiled broadcast directly at those partitions."""
                usb = ph2.tile([65, 1024], BF16, tag="usb", bufs=GRP + 2, name=f"usb{h}")
                nc.vector.tensor_copy(usb[:, :], u[0:65, :])
                if stage:
                    # stage this head's S row (bf16) for the group Ln/Exp
                    nc.sync.dma_start(sT[h % GRP : h % GRP + 1, :], usb[64:65, :])
                usbs[h] = usb
                if h % 2 == 1:
                    uhi = ph2.tile([128, 1024], BF16, tag="uhi", bufs=2, name=f"uhi{h}")
                    nc.vector.stream_shuffle(
                        uhi[64:128, :], usb[0:64, :], list(range(32))
                    )
                    uhis[h] = uhi

            def nv_store(h, bc):
                """nv = usb * (1/S broadcast). Even heads read bc rows 0:64,
                odd heads read rows 64:128 (their usb was pre-shuffled)."""
                nvv = nvP8[h // 4][:, :].rearrange("p (j n) -> p j n", j=2)
                if h % 2 == 0:
                    usb = usbs.pop(h)
                    nc.vector.tensor_mul(
                        nvv[0:64, (h // 2) % 2, :], usb[0:64, :], bc[0:64, :]
                    )
                else:
                    usbs.pop(h)
                    uhi = uhis.pop(h)
                    nc.vector.tensor_mul(
                        nvv[64:128, (h // 2) % 2, :], uhi[64:128, :], bc[64:128, :]
                    )

            def bc_pair(he, ho, ke, ko):
                """One psum tile broadcasts 1/S for an (even, odd) head pair:
                even at out partitions 0:64, odd col-tiled to 64:128."""
                bc = pmm.tile([128, 1024], F32, tag="mm", name=f"bc{he}")
                for p0, k in ((0, ke), (64, ko)):
                    for wc in range(2):
                        nc.tensor.matmul(
                            bc[p0 : p0 + 64, wc * 512 : (wc + 1) * 512],
                            ones[0:1, 0:64],
                            rcbF[0:1, k * 1024 + wc * 512 : k * 1024 + (wc + 1) * 512],
                            start=True,
                            stop=True,
                        )
                nv_store(he, bc)
                nv_store(ho, bc)

            def make_group_inject(heads):
                """Batched 1/S for `heads`, interleaved into the next head's
                streams: Ln+Exp after pair 1 (S-row DMAs long done), the
                broadcast matmuls + nv multiplies after pairs 3 and 5."""
                r1 = len(heads)

                def ln_exp():
                    nc.scalar.activation(lnT[0:r1, :], sT[0:r1, :], AF.Ln)
                    nc.scalar.activation(rcbT[0:r1, :], lnT[0:r1, :], AF.Exp, scale=-1.0)
                    nc.sync.dma_start(rcbF[0:1, 0 : r1 * 1024], rcbT[0:r1, :])

                inj = {2: [ln_exp]}
                for k in range(0, len(heads), 2):
                    inj.setdefault(4 + k, []).append(
                        lambda he=heads[k], ho=heads[k + 1], ke=k, ko=k + 1: bc_pair(
                            he, ho, ke, ko
                        )
                    )
                return inj

            def direct_ln_exp(h):
                """v1-style single-head 1/S (no DMA staging) for the last
                heads, where chain latency matters more than ACT stream time."""
                usb = usbs[h]
                lnd = ph2.tile([65, 1024], F32, tag="lnd", bufs=1, name=f"lnd{h}")
                rcd = ph2.tile([65, 1024], BF16, tag="rcd", bufs=1, name=f"rcd{h}")
                nc.scalar.activation(lnd[64:65, :], usb[64:65, :], AF.Ln)
                nc.scalar.activation(rcd[64:65, :], lnd[64:65, :], AF.Exp, scale=-1.0)
                direct_rcb[h] = rcd

            def direct_bc(h):
                rcd = direct_rcb.pop(h)
                p0 = 0 if h % 2 == 0 else 64
                bc = pmm.tile([128, 1024], F32, tag="mm", name=f"bcd{h}")
                for wc in range(2):
                    mm = nc.tensor.matmul(
                        bc[p0 : p0 + 64, wc * 512 : (wc + 1) * 512],
                        ones[64:65, 0:64],
                        rcd[64:65, wc * 512 : (wc + 1) * 512],
                        start=True,
                        stop=True,
                    )
                    if wc == 1:
                        mm.ins.ldweights = False
                nv_store(h, bc)

            # ---- all q/k projections upfront (PE-bound, silu table resident)
            for et in range(ND):
                project_qk(et)
                project_qk(ND + et)
                if et + 2 < ND:
                    fetch_wqk(et + 2)
                    fetch_wqk(ND + et + 2)

            # ---- pure-attention loop (exp table resident)
            # prefetch the residual x rows now: the gpsimd DMA queue is idle
            # for the whole attention phase and phase 3 needs them at once
            xrs = []
            for wt in range(NW):
                xr = qkv_pool.tile([128, 1024], F32, tag=f"xr{wt}", name=f"xr{wt}")
                nc.gpsimd.dma_start(xr[:, :], xq_d[wt * 128 : (wt + 1) * 128, :])
                xrs.append(xr)
            inject_next = {}
            for h in range(H):
                u = attn_mms(h, inject_next)
                inject_next = {}
                normalize_a(h, u, stage=(h < 14))
                if h in (3, 7, 11):
                    inject_next = make_group_inject(list(range(h - 3, h + 1)))
                elif h == 13:
                    inject_next = make_group_inject([12, 13])
                elif h == 14:
                    inject_next = {2: [lambda: direct_ln_exp(14)], 4: [lambda: direct_bc(14)]}
            # head 15: shortest-latency direct chain in the tail
            direct_ln_exp(15)
            direct_bc(15)
            # dummy silu: hoists the ACT silu-table load ahead of the first
            # output-projection psum (the load costs 1.3us on the tail path)
            dum = ph2.tile([1, 8], F32, tag="dum")
            nc.scalar.activation(dum[0:1, :], lnT[0:1, 0:8], AF.Silu)

        # ---------------- phase 3: output projection + LN ------------------
        with tc.tile_pool(name="ph3", bufs=1) as ph3:
            mvall = ph3.tile([128, 2 * NW], F32, tag="mvall")
            sd = ph3.tile([128, 2 * NW], F32, tag="sd")
            ys = []
            w28v3 = w28[:, :].rearrange("p (e c) -> p e c", e=8)
            nvviews = [t[:, :].rearrange("p (j n) -> p j n", j=2) for t in nvP8]

            def outproj_stats(wt):
                po = pmm.tile([128, 1024], F32, tag="mm")
                for ep in range(4):
                    for dc in range(2):
                        sl = slice(dc * 512, (dc + 1) * 512)
                        mm = nc.tensor.matmul(
                            po[:, sl],
                            nvviews[ep][:, :, wt * 128 : (wt + 1) * 128],
                            w28v3[:, 2 * ep : 2 * ep + 2, sl],
                            start=(ep == 0),
                            stop=False,
                            perf_mode=PM.DoubleRow,
                        )
                        if dc == 1:
                            mm.ins.ldweights = False
                for dc in range(2):
                    sl = slice(dc * 512, (dc + 1) * 512)
                    mm = nc.tensor.matmul(
                        po[:, sl],
                        ones[0:1, 0:128],
                        b2[0:1, sl],
                        start=False,
                        stop=True,
                    )
                    if dc == 1:
                        mm.ins.ldweights = False
                msb = ph3.tile([128, 1024], F32, tag="m", bufs=2)
                nc.scalar.activation(msb[:, :], po[:, :], AF.Silu)
                y = xrs[wt]
                nc.vector.tensor_add(y[:, :], msb[:, :], y[:, :])
                ys.append(y)
                st = ph3.tile([128, 12], F32, tag="st", bufs=2)
                nc.vector.bn_stats(st[:, 0:6], y[:, 0:512])
                nc.vector.bn_stats(st[:, 6:12], y[:, 512:1024])
                nc.vector.bn_aggr(mvall[:, 2 * wt : 2 * wt + 2], st[:, :])

            def ln_batch(wts):
                w0, w1 = wts[0], wts[-1] + 1
                nc.scalar.activation(
                    sd[:, w0:w1],
                    mvall[:, 2 * w0 + 1 : 2 * w1 : 2],
                    AF.Sqrt,
                    bias=eps[:, 0:1],
                )
                nc.vector.reciprocal(sd[:, NW + w0 : NW + w1], sd[:, w0:w1])
                for wt in wts:
                    ot = ph3.tile([128, 1024], BF16, tag="ot", bufs=2)
                    nc.vector.tensor_scalar(
                        ot[:, :],
                        ys[wt][:, :],
                        mvall[:, 2 * wt : 2 * wt + 1],
                        sd[:, NW + wt : NW + wt + 1],
                        ALU.subtract,
                        ALU.mult,
                    )
                    nc.sync.dma_start(out_d[wt * 128 : (wt + 1) * 128, :], ot[:, :])

            for wt in range(2):
                outproj_stats(wt)
            ln_batch([0, 1])
            for wt in range(2, 4):
                outproj_stats(wt)
            ln_batch([2, 3])
            for wt in range(4, 6):
                outproj_stats(wt)
            ln_batch([4, 5])
            for wt in range(6, NW):
                outproj_stats(wt)
            ln_batch([6, 7])

        qkv_pool.release()
        pu.release()
        pmm.release()
        pers.release()

    if split_waits:
        _split_excess_waits(nc)
    return nc


_NC_CACHE = None


def _get_program():
    global _NC_CACHE
    if _NC_CACHE is None:
        _NC_CACHE = build_program()
    return _NC_CACHE


def _pretile_weights(W_fc, b_fc, W_fc2, b_fc2):
    """Host-side: build the exact fp8/bf16 tile layouts the kernel DMAs."""
    W_fc = np.asarray(W_fc, dtype=np.float32).reshape(D, H, 3 * HD)
    b_fc = np.asarray(b_fc, dtype=np.float32).reshape(H, 3 * HD)
    W_fc2 = np.asarray(W_fc2, dtype=np.float32)
    b_fc2 = np.asarray(b_fc2, dtype=np.float32)

    def to8(a):
        return np.clip(a, -240, 240).astype(F8)

    # wqk[et, p, kd*128 + hl*64 + c] = W_fc[kd*128+p, 2*(et%8)+hl, c0+c]
    wqk = np.empty((16, 128, 1024), dtype=F8)
    for et in range(16):
        is_q = et < 8
        qi = et % 8
        c0 = 0 if is_q else HD
        blk = W_fc[:, 2 * qi : 2 * qi + 2, c0 : c0 + HD].reshape(8, 128, 128)
        wqk[et] = to8(blk.transpose(1, 0, 2).reshape(128, 1024))

    # wv[p, kd*1024 + h*64 + c] = W_fc[kd*128+p, h, 128+c]
    wv = to8(
        W_fc[:, :, 2 * HD : 3 * HD].reshape(8, 128, H * HD).transpose(1, 0, 2).reshape(128, 8 * 1024)
    )

    # w2[p, e*1024 + d] = W_fc2[e*128+p, d]
    w2 = to8(W_fc2.reshape(8, 128, D).transpose(1, 0, 2).reshape(128, 8 * 1024))

    bqk = np.empty((128, 16), dtype=np.float32)
    for et in range(16):
        is_q = et < 8
        qi = et % 8
        c0 = 0 if is_q else HD
        bqk[:, et] = b_fc[2 * qi : 2 * qi + 2, c0 : c0 + HD].reshape(128)

    bv = b_fc[:, 2 * HD : 3 * HD].reshape(1, H * HD).astype(BF)
    b2 = b_fc2.reshape(1, D).astype(BF)
    return wqk, wv, w2, bqk, bv, b2


def make_in_maps(x, W_fc, b_fc, W_fc2, b_fc2):
    x = np.asarray(x, dtype=np.float32)
    wqk, wv, w2, bqk, bv, b2 = _pretile_weights(W_fc, b_fc, W_fc2, b_fc2)
    in_maps = []
    for i in range(N_CORES):
        b = i // 2
        w0 = (i % 2) * WQ
        xrot = np.concatenate([x[b, w0:], x[b, :w0]], axis=0)
        # x8[p, kd*L + t] = xrot[t, kd*128+p]
        xT = xrot.T.reshape(8, 128, L)
        x8 = np.clip(xT.transpose(1, 0, 2).reshape(128, 8 * L), -240, 240).astype(F8)
        xq = np.ascontiguousarray(x[b, w0 : w0 + WQ])
        in_maps.append(
            {
                "x8": np.ascontiguousarray(x8),
                "xq": xq,
                "wqk": wqk,
                "wv": wv,
                "w2": w2,
                "bqk": bqk,
                "bv": bv,
                "b2": b2,
            }
        )
    return in_maps


def kernel(x, W_fc, b_fc, W_fc2, b_fc2, **extra):
    nc = _get_program()
    in_maps = make_in_maps(x, W_fc, b_fc, W_fc2, b_fc2)
    res = run_bass_kernel_spmd(nc, in_maps, list(range(N_CORES)))
    out = np.empty((B, L, D), dtype=np.float32)
    for i in range(N_CORES):
        b = i // 2
        w0 = (i % 2) * WQ
        out[b, w0 : w0 + WQ] = res.results[i]["out"].astype(np.float32)
    return out
